# revision 1
# baseline (speedup 1.0000x reference)
"""ChebNet (K=3) GNN message passing on 8 Trainium2 NeuronCores.

Strategy (graph-partition parallelism): destination nodes are sharded
across the 8 cores.  Each core's nodes are re-packed into 100 tiles of
128 "slots" balancing per-tile edge counts; edges are grouped by
(dst tile, src chunk) into 128-lane blocks.  On device, each block's
source rows are fetched with the SWDGE dma_gather engine (int16 indices,
4 table chunks spread over 4 SWDGE queues), a weighted one-hot
(iota == slot) * w is built in one DVE op, and a TensorEngine matmul
accumulates the segment sum in PSUM:

    q1 = Q(x)         where Q(h)[i] = sum_{e: dst=i} dinv[src]dinv[dst] h[src]
    q2 = Q(q1)        (kernel 2; q1 relayed through the host between NEFFs)
    out = log_softmax(relu(x(W0-W2) - q1 W1 + 2 q2 W2 + bias) @ fcW + fcb)

Math identity used: with unit edge weights lambda_max == 2 exactly, so the
rescaled Laplacian is -D^-1/2 A D^-1/2 (loop weights cancel), giving
Tx1 = -q1 and Tx2 = 2*q2 - x.

All feature data moves in bf16 (tables, matmul operands) with fp32 PSUM
accumulation and an fp32 softmax tail; end-to-end relative error vs the
fp32 reference is ~1e-3.
"""

import os

os.environ.setdefault("MYCRO_LOCAL_CACHE", "1")

from contextlib import ExitStack

import numpy as np
import ml_dtypes

import concourse.bass as bass
import concourse.tile as tile
from concourse import bacc, mybir
from concourse.bass_utils import run_bass_kernel_spmd

F32 = mybir.dt.float32
BF16 = mybir.dt.bfloat16
I16 = mybir.dt.int16
AX = mybir.AxisListType
OP = mybir.AluOpType
AF = mybir.ActivationFunctionType

# ---- hardcoded problem geometry ------------------------------------------
N = 100_000          # nodes
F = 128              # features
NCLS = 40            # classes
NCORES = 8
NCHUNK = 4           # int16 index range forces 4 table chunks
TILES = 100          # dst tiles per core
SG = 10              # dst tiles per gather supergroup
KC = (4, 4, 4, 4)    # blocks per (tile, chunk)
B = sum(KC)
KCOFF = (0, 4, 8, 12)
NPC = N // NCORES            # 12500 real nodes per core
NPCP = TILES * 128           # 12800 padded rows per core
NBLK = TILES * B             # 1600 blocks per core
NTAB = NCORES * NPCP         # 102400 rows in the q1 table
CHN_X = N // NCHUNK          # 25000 x-table rows per chunk
CHN_Q = NTAB // NCHUNK       # 25600 q1-table rows per chunk
TUNE1 = dict(stage_bufs=3, oh_bufs=6, ps_bufs=4, evac_act=True)


# ---- host-side graph prep -------------------------------------------------

def _pack_core(dloc, echunk):
    """Assign local nodes to (tile, slot), balancing per-(tile,chunk) load."""
    cntc = np.zeros((NPC, NCHUNK), np.int64)
    np.add.at(cntc, (dloc, echunk), 1)
    order = np.argsort(-cntc.sum(1), kind="stable")
    load = np.zeros((TILES, NCHUNK), np.float64)
    fill = np.zeros(TILES, np.int32)
    target = np.maximum(cntc.sum(0) / TILES, 1.0)
    tile_of = np.empty(NPC, np.int32)
    slot_of = np.empty(NPC, np.int32)
    for node in order:
        score = ((load + cntc[node]) / target).max(1)
        score[fill >= 128] = np.inf
        t = int(score.argmin())
        tile_of[node] = t
        slot_of[node] = fill[t]
        fill[t] += 1
        load[t] += cntc[node]
    return tile_of, slot_of, load.astype(np.int64)


def _wrap_idx(local_rows, kc):
    """local_rows [TILES, kc, 128] -> wrapped int16 [128, TILES*kc*8]."""
    cols_per_call = SG * kc * 8
    out = np.zeros((16, TILES * kc * 8), np.int16)
    for g0 in range(0, TILES, SG):
        gs = min(SG, TILES - g0)
        flat = local_rows[g0:g0 + gs].reshape(-1)
        j = np.arange(flat.size)
        out[j % 16, (g0 // SG) * cols_per_call + j // 16] = flat
    return np.ascontiguousarray(np.tile(out, (8, 1)).astype(np.int16))


def _prep(edge_index):
    src = np.asarray(edge_index[0], dtype=np.int64)
    dst = np.asarray(edge_index[1], dtype=np.int64)
    nonloop = src != dst
    deg = np.bincount(src[nonloop], minlength=N).astype(np.float32)
    dinv = np.where(deg > 0, deg ** -0.5, 0.0).astype(np.float32)
    w = np.where(nonloop, dinv[src] * dinv[dst], 0.0).astype(np.float32)
    keep = w > 0
    src, dst, w = src[keep], dst[keep], w[keep]

    owner_dst = dst // NPC
    echunk_all = (src // NPC) // (NCORES // NCHUNK)

    tile_slot, ppos = [], []
    for c in range(NCORES):
        sel = owner_dst == c
        dloc = (dst[sel] - c * NPC).astype(np.int64)
        tile_of, slot_of, load = _pack_core(dloc, echunk_all[sel])
        assert (-(-load.max(0) // 128) <= np.array(KC)).all(), \
            "per-(tile,chunk) load exceeds KC capacity"
        tile_slot.append((tile_of, slot_of))
        ppos.append((tile_of * 128 + slot_of).astype(np.int32))

    gpos = np.empty(N, np.int64)
    for c in range(NCORES):
        gpos[c * NPC:(c + 1) * NPC] = c * NPCP + ppos[c]

    koff = np.asarray(KCOFF)
    dl_l, w_l, idx1_l, idx2_l = [], [], [], []
    for c in range(NCORES):
        sel = owner_dst == c
        es, ew, ec = src[sel], w[sel], echunk_all[sel]
        dloc = (dst[sel] - c * NPC).astype(np.int64)
        tile_of, slot_of = tile_slot[c]
        et = tile_of[dloc]
        eslot = slot_of[dloc]
        key = et * NCHUNK + ec
        order = np.argsort(key, kind="stable")
        es, ew, et, ec, eslot = (es[order], ew[order], et[order], ec[order],
                                 eslot[order])
        counts = np.bincount(key[order], minlength=TILES * NCHUNK)
        starts = np.concatenate([[0], np.cumsum(counts)[:-1]])
        within = np.arange(len(es)) - starts[key[order]]
        col = et * B + koff[ec] + within // 128
        lane = within % 128
        s1 = np.zeros((128, NBLK), np.int64)
        s2 = np.zeros((128, NBLK), np.int64)
        dlv = np.zeros((128, NBLK), np.float32)
        wv = np.zeros((128, NBLK), np.float32)
        for cc in range(NCHUNK):
            cols = np.add.outer(np.arange(TILES) * B,
                                koff[cc] + np.arange(KC[cc])).ravel()
            s1[:, cols] = cc * CHN_X
            s2[:, cols] = cc * CHN_Q
        s1[lane, col] = es
        s2[lane, col] = gpos[es]
        dlv[lane, col] = eslot.astype(np.float32)
        wv[lane, col] = ew
        dl_l.append(dlv)
        w_l.append(wv)
        i1c, i2c = [], []
        for cc in range(NCHUNK):
            cols = np.add.outer(np.arange(TILES) * B,
                                koff[cc] + np.arange(KC[cc]))
            loc1 = s1[:, cols] - cc * CHN_X
            loc2 = s2[:, cols] - cc * CHN_Q
            i1c.append(_wrap_idx(loc1.transpose(1, 2, 0), KC[cc]))
            i2c.append(_wrap_idx(loc2.transpose(1, 2, 0), KC[cc]))
        idx1_l.append(i1c)
        idx2_l.append(i2c)
    return dl_l, w_l, idx1_l, idx2_l, ppos


# ---- bass kernel builders -------------------------------------------------

def _propagate(nc, ctx, tc, tab, chn, idx_drams, dl_sb, w_sb, iota_sb,
               per_tile_emit, feature_major, tune):
    stage_pool = ctx.enter_context(
        tc.tile_pool(name="stage", bufs=tune.get("stage_bufs", 2)))
    idx_pool = ctx.enter_context(tc.tile_pool(name="idxp", bufs=2))
    oh_pool = ctx.enter_context(
        tc.tile_pool(name="oh", bufs=tune.get("oh_bufs", 3)))
    psum_pool = ctx.enter_context(
        tc.tile_pool(name="psq", bufs=tune.get("ps_bufs", 2), space="PSUM"))

    for g0 in range(0, TILES, SG):
        gs = min(SG, TILES - g0)
        stages = []
        for cc in range(NCHUNK):
            kc = KC[cc]
            ni = gs * kc * 128
            ic = gs * kc * 8
            idx_sb = idx_pool.tile([128, SG * kc * 8], I16, tag=f"idx{cc}")
            co = (g0 // SG) * SG * kc * 8
            nc.sync.dma_start(idx_sb[:, :ic],
                              idx_drams[cc].ap()[:, co:co + ic])
            stage = stage_pool.tile([128, SG * kc, F], BF16, tag=f"stage{cc}")
            nc.gpsimd.dma_gather(
                out_ap=stage[:, :gs * kc, :],
                in_ap=tab[cc * chn:(cc + 1) * chn, :],
                idxs_ap=idx_sb[:, :ic],
                num_idxs=ni,
                num_idxs_reg=ni,
                elem_size=F,
                single_packet=False,
                queue_num=cc,
            )
            stages.append(stage)
        for ti in range(gs):
            t = g0 + ti
            psum = psum_pool.tile([128, 128], F32, tag="psum_q")
            bi = 0
            for cc in range(NCHUNK):
                for b in range(KC[cc]):
                    col = t * B + KCOFF[cc] + b
                    oh = oh_pool.tile([128, 128], BF16, tag="oh")
                    nc.vector.tensor_scalar(
                        out=oh[:], in0=iota_sb[:],
                        scalar1=dl_sb[:, col:col + 1],
                        scalar2=w_sb[:, col:col + 1],
                        op0=OP.is_equal, op1=OP.mult)
                    g = stages[cc][:, ti * KC[cc] + b, :]
                    if feature_major:
                        nc.tensor.matmul(psum[:], g, oh[:],
                                         start=(bi == 0), stop=(bi == B - 1))
                    else:
                        nc.tensor.matmul(psum[:], oh[:], g,
                                         start=(bi == 0), stop=(bi == B - 1))
                    bi += 1
            per_tile_emit(t, psum)


def _common_inputs(nc, idx_name):
    idxd = [nc.dram_tensor(f"{idx_name}{cc}", [128, TILES * KC[cc] * 8], I16,
                           kind="ExternalInput") for cc in range(NCHUNK)]
    dl = nc.dram_tensor("dl", [128, NBLK], F32, kind="ExternalInput")
    w = nc.dram_tensor("w", [128, NBLK], F32, kind="ExternalInput")
    iota = nc.dram_tensor("iota", [128, 128], BF16, kind="ExternalInput")
    return idxd, dl, w, iota


def _load_const(nc, pool, pairs):
    out = []
    for dr in pairs:
        sb = pool.tile(list(dr.shape), dr.dtype, tag=f"c_{dr.name}")
        nc.sync.dma_start(sb[:], dr.ap()[:, :])
        out.append(sb)
    return out


def build_kernel1():
    nc = bacc.Bacc("TRN2", target_bir_lowering=False, debug=False,
                   num_devices=NCORES, num_swdge_queues=4)
    tab = nc.dram_tensor("x_tab", [N, F], BF16, kind="ExternalInput")
    idxd, dl, w, iota = _common_inputs(nc, "idx")
    q1 = nc.dram_tensor("q1", [NPCP, F], F32, kind="ExternalOutput")

    with tile.TileContext(nc) as tc, ExitStack() as ctx:
        const_pool = ctx.enter_context(tc.tile_pool(name="const", bufs=1))
        evac_pool = ctx.enter_context(tc.tile_pool(name="evac", bufs=3))
        dl_sb, w_sb, iota_sb = _load_const(nc, const_pool, (dl, w, iota))

        def emit(t, psum):
            out_sb = evac_pool.tile([128, F], F32, tag="evac")
            nc.scalar.copy(out_sb[:], psum[:])
            nc.sync.dma_start(q1.ap()[t * 128:(t + 1) * 128, :], out_sb[:])

        _propagate(nc, ctx, tc, tab.ap(), CHN_X, idxd, dl_sb, w_sb, iota_sb,
                   emit, feature_major=False, tune=TUNE1)
    nc.compile()
    return nc


def build_kernel2():
    nc = bacc.Bacc("TRN2", target_bir_lowering=False, debug=False,
                   num_devices=NCORES, num_swdge_queues=4)
    tab = nc.dram_tensor("q1_tab", [NTAB, F], BF16, kind="ExternalInput")
    idxd, dl, w, iota = _common_inputs(nc, "jdx")
    xT = nc.dram_tensor("xT", [128, NPCP], BF16, kind="ExternalInput")
    q1T = nc.dram_tensor("q1T", [128, NPCP], BF16, kind="ExternalInput")
    A = nc.dram_tensor("A", [F, 128], BF16, kind="ExternalInput")
    Bm = nc.dram_tensor("Bm", [F, 128], BF16, kind="ExternalInput")
    Cm = nc.dram_tensor("Cm", [F, 128], BF16, kind="ExternalInput")
    bias = nc.dram_tensor("bias", [128, 1], F32, kind="ExternalInput")
    fcW = nc.dram_tensor("fcW", [128, NCLS], BF16, kind="ExternalInput")
    fcb = nc.dram_tensor("fcb", [128, NCLS], F32, kind="ExternalInput")
    out = nc.dram_tensor("out", [NPCP, NCLS], F32, kind="ExternalOutput")

    with tile.TileContext(nc) as tc, ExitStack() as ctx:
        const_pool = ctx.enter_context(tc.tile_pool(name="const", bufs=1))
        work_pool = ctx.enter_context(tc.tile_pool(name="work", bufs=3))
        small_pool = ctx.enter_context(tc.tile_pool(name="small", bufs=4))
        psum_o_pool = ctx.enter_context(
            tc.tile_pool(name="psum_o", bufs=2, space="PSUM"))
        psum_l_pool = ctx.enter_context(
            tc.tile_pool(name="psum_l", bufs=2, space="PSUM"))
        (dl_sb, w_sb, iota_sb, A_sb, B_sb, C_sb, bias_sb, fcW_sb,
         fcb_sb) = _load_const(nc, const_pool,
                               (dl, w, iota, A, Bm, Cm, bias, fcW, fcb))

        def emit(t, psum_q2T):
            ts = slice(t * 128, (t + 1) * 128)
            xT_t = work_pool.tile([128, 128], BF16, tag="xT_t")
            nc.sync.dma_start(xT_t[:], xT.ap()[:, ts])
            q1T_t = work_pool.tile([128, 128], BF16, tag="q1T_t")
            nc.sync.dma_start(q1T_t[:], q1T.ap()[:, ts])
            q2T_sb = work_pool.tile([128, 128], BF16, tag="q2T")
            nc.scalar.copy(q2T_sb[:], psum_q2T[:])

            psum_o = psum_o_pool.tile([128, 128], F32, tag="psum_o")
            nc.tensor.matmul(psum_o[:], A_sb[:], xT_t[:],
                             start=True, stop=False)
            nc.tensor.matmul(psum_o[:], B_sb[:], q1T_t[:],
                             start=False, stop=False)
            nc.tensor.matmul(psum_o[:], C_sb[:], q2T_sb[:],
                             start=False, stop=True)
            hT = work_pool.tile([128, 128], BF16, tag="hT")
            nc.scalar.activation(hT[:], psum_o[:], AF.Relu,
                                 bias=bias_sb[:, 0:1], scale=1.0)

            psum_l = psum_l_pool.tile([128, NCLS], F32, tag="psum_l")
            nc.tensor.matmul(psum_l[:], hT[:], fcW_sb[:],
                             start=True, stop=True)
            lb = small_pool.tile([128, NCLS], F32, tag="lb")
            nc.vector.tensor_tensor(out=lb[:], in0=psum_l[:], in1=fcb_sb[:],
                                    op=OP.add)
            negm = small_pool.tile([128, 1], F32, tag="negm")
            nc.vector.tensor_reduce(negm[:], lb[:], axis=AX.X, op=OP.max,
                                    negate=True)
            esc = small_pool.tile([128, NCLS], F32, tag="esc")
            ssum = small_pool.tile([128, 1], F32, tag="ssum")
            nc.scalar.activation(esc[:], lb[:], AF.Exp, bias=negm[:, 0:1],
                                 scale=1.0, accum_out=ssum[:, 0:1])
            logs = small_pool.tile([128, 1], F32, tag="logs")
            nc.scalar.activation(logs[:], ssum[:], AF.Ln)
            out_sb = small_pool.tile([128, NCLS], F32, tag="out_sb")
            nc.vector.tensor_scalar(out=out_sb[:], in0=lb[:],
                                    scalar1=negm[:, 0:1],
                                    scalar2=logs[:, 0:1],
                                    op0=OP.add, op1=OP.subtract)
            nc.sync.dma_start(out.ap()[ts, :], out_sb[:])

        _propagate(nc, ctx, tc, tab.ap(), CHN_Q, idxd, dl_sb, w_sb, iota_sb,
                   emit, feature_major=True, tune={})
    nc.compile()
    return nc


_CACHE = {}


def _bf16(a):
    return np.asarray(a).astype(ml_dtypes.bfloat16)


def kernel(x, edge_index, W0, W1, W2, cheb_bias, fc_W, fc_b):
    x = np.asarray(x)
    dl_l, w_l, idx1_l, idx2_l, ppos = _prep(np.asarray(edge_index))

    if "nc1" not in _CACHE:
        _CACHE["nc1"] = build_kernel1()
        _CACHE["nc2"] = build_kernel2()
    nc1, nc2 = _CACHE["nc1"], _CACHE["nc2"]

    iota = np.ascontiguousarray(
        np.broadcast_to(np.arange(128, dtype=np.float32),
                        (128, 128))).astype(ml_dtypes.bfloat16)
    x_tab = _bf16(x)
    maps1 = []
    for c in range(NCORES):
        m = dict(x_tab=x_tab, dl=dl_l[c], w=w_l[c], iota=iota)
        for cc in range(NCHUNK):
            m[f"idx{cc}"] = idx1_l[c][cc]
        maps1.append(m)
    res1 = run_bass_kernel_spmd(nc1, maps1, core_ids=list(range(NCORES)))
    q1 = [res1.results[c]["q1"] for c in range(NCORES)]

    q1_tab = _bf16(np.concatenate(q1, 0))
    A_b = _bf16(np.asarray(W0, np.float32) - np.asarray(W2, np.float32))
    B_b = _bf16(-np.asarray(W1, np.float32))
    C_b = _bf16(2.0 * np.asarray(W2, np.float32))
    fcW_b = _bf16(fc_W)
    fcb_rep = np.ascontiguousarray(
        np.broadcast_to(np.asarray(fc_b, np.float32), (128, NCLS)))
    bias_col = np.ascontiguousarray(
        np.asarray(cheb_bias, np.float32).reshape(128, 1))
    maps2 = []
    for c in range(NCORES):
        xperm = np.zeros((NPCP, F), np.float32)
        xperm[ppos[c]] = x[c * NPC:(c + 1) * NPC]
        m = dict(q1_tab=q1_tab, dl=dl_l[c], w=w_l[c], iota=iota,
                 xT=np.ascontiguousarray(_bf16(xperm).T),
                 q1T=np.ascontiguousarray(_bf16(q1[c]).T),
                 A=A_b, Bm=B_b, Cm=C_b, bias=bias_col, fcW=fcW_b,
                 fcb=fcb_rep)
        for cc in range(NCHUNK):
            m[f"jdx{cc}"] = idx2_l[c][cc]
        maps2.append(m)
    res2 = run_bass_kernel_spmd(nc2, maps2, core_ids=list(range(NCORES)))

    out = np.zeros((N, NCLS), np.float32)
    for c in range(NCORES):
        out[c * NPC:(c + 1) * NPC] = res2.results[c]["out"][ppos[c]]
    return out
